# revision 71
# baseline (speedup 1.0000x reference)
"""Sliding-window GQA causal self-attention block for 8 trn2 NeuronCores.

Sharding: batch (4) x T-halves (2) -> 8 cores, no collectives. Each core gets
x.T for its T-half plus a 256-row key/value halo and computes its (1024, 1024)
slice of the output.

Schedule (single continuous PE stream, cost-model: matmul = out-free-size
rows):
  A: k-proj then v-proj (fp8 hi/lo DoubleRow: 6N rows per C-contraction vs
     8N bf16), k-rope + kT SBUF-DMAs overlapped with v-proj.
  B: q-proj per (t-half, c4) in [128,512] psum chunks, rope, qT DMAs.
  C: 64-slot pipeline over (qb, head-pair): scores (keys on partitions,
     3x128 cols) -> exp on ACT (2 heads batched, scale=1/8, no max-sub) ->
     band-edge masks (gpsimd affine_select for the lower edge, DVE mul for
     the causal edge) -> att@v with queries on psum partitions (N=65/call,
     ones column gives softmax denominators, 3-slot lag) -> reciprocal +
     one stride-0-broadcast normalize mul on DVE -> PE transpose (identity
     matmul) into spare bytes of a st tile -> output projection spread 4
     matmuls/slot into dedicated psum banks -> psum->sbuf on ACT+DVE, DMA
     out (split across sync/gpsimd queues).
PSUM: st [128,962]f32 x3 bufs = 6 banks (cols 0:768 scores, 768:898 att@v
accumulators written two slots ahead, 898:962 bf16 transpose target) +
op0/op1 [128,512]f32 = 2 banks. GPSIMD never touches PSUM (hw illegal);
DMA never touches PSUM (bass asserts).
"""

import dataclasses

import numpy as np
import ml_dtypes

import concourse.bass as bass
import concourse.mybir as mybir
import concourse.tile as tile
from concourse import bacc
from concourse.bass_utils import run_bass_kernel_spmd

BF = ml_dtypes.bfloat16
F8 = ml_dtypes.float8_e4m3
F32 = mybir.dt.float32
BF16 = mybir.dt.bfloat16
FP8 = mybir.dt.float8e4

B, T, C = 4, 2048, 1024
H, KV, HD = 16, 4, 64
WIN = 256
TL = T // 2            # 1024 own rows per core
TH = TL + WIN          # 1280 with halo
NEG = -30000.0

USE_FP8 = True
SX = 4.0               # fp8 scale for x
SW = 64.0              # fp8 scale for w_attn (qk and v slices)
DESCALE = 1.0 / (SX * SW)

DR = mybir.MatmulPerfMode.DoubleRow


def _build_program():
    nc = bacc.Bacc("TRN2", target_bir_lowering=False, debug=False, num_devices=8)
    dt = mybir.dt
    if USE_FP8:
        # x: 3-slot (hi,hi,lo); weights: 2-slot (hi,lo) along the pair dim
        xf = nc.dram_tensor("xf", [8, 128, 2, TH], dt.float8e4, kind="ExternalInput").ap()
        wq = nc.dram_tensor("wq", [128, 8, 2, 1024], dt.float8e4, kind="ExternalInput").ap()
        wk = nc.dram_tensor("wk", [128, 8, 2, 256], dt.float8e4, kind="ExternalInput").ap()
        wv = nc.dram_tensor("wv", [128, 8, 2, 256], dt.float8e4, kind="ExternalInput").ap()
    else:
        xf = nc.dram_tensor("xf", [8, 128, TH], dt.bfloat16, kind="ExternalInput").ap()
        wq = nc.dram_tensor("wq", [128, 8, 1024], dt.bfloat16, kind="ExternalInput").ap()
        wk = nc.dram_tensor("wk", [128, 8, 256], dt.bfloat16, kind="ExternalInput").ap()
        wv = nc.dram_tensor("wv", [128, 8, 256], dt.bfloat16, kind="ExternalInput").ap()
    wp = nc.dram_tensor("wp", [128, 8, C], dt.bfloat16, kind="ExternalInput").ap()
    cq = nc.dram_tensor("cq", [128, 2, TL], dt.bfloat16, kind="ExternalInput").ap()
    ck = nc.dram_tensor("ck", [128, 2, TH], dt.bfloat16, kind="ExternalInput").ap()
    vb = nc.dram_tensor("vb", [1, 640], dt.bfloat16, kind="ExternalInput").ap()
    out = nc.dram_tensor("out", [TL, C], dt.float32, kind="ExternalOutput").ap()

    with tile.TileContext(nc) as tc:
        _kernel_body(tc, nc, xf, wq, wk, wv, wp, cq, ck, vb, out)
    nc.compile()
    return nc


def _kernel_body(tc, nc, xf, wq, wk, wv, wp, cq, ck, vb, out):
    import contextlib
    ctx = contextlib.ExitStack()
    with ctx:
        consts = ctx.enter_context(tc.tile_pool(name="consts", bufs=1))
        persist = ctx.enter_context(tc.tile_pool(name="persist", bufs=1))

        # ---- persistent inputs (DMA order = consumption order) ----
        if USE_FP8:
            x_sb = persist.tile([128, 8, 2, TH], FP8, tag="x")
            wq_sb = persist.tile([128, 8, 2, 1024], FP8, tag="wq")
            wk_sb = persist.tile([128, 8, 2, 256], FP8, tag="wk")
            wv_sb = persist.tile([128, 8, 2, 256], FP8, tag="wv")
        else:
            x_sb = persist.tile([128, 8, TH], BF16, tag="x")
            wq_sb = persist.tile([128, 8, 1024], BF16, tag="wq")
            wk_sb = persist.tile([128, 8, 256], BF16, tag="wk")
            wv_sb = persist.tile([128, 8, 256], BF16, tag="wv")
        def wq_piece(idx, eng):
            c4, half = divmod(idx, 2)
            c0 = half * 512 + c4 * 128
            if USE_FP8:
                eng.dma_start(out=wq_sb[:, :, :, c0:c0 + 128],
                              in_=wq[:, :, :, c0:c0 + 128])
            else:
                eng.dma_start(out=wq_sb[:, :, c0:c0 + 128],
                              in_=wq[:, :, c0:c0 + 128])

        if USE_FP8:
            nc.sync.dma_start(out=x_sb[:, 0, 0], in_=xf[0, :, 0])
            nc.gpsimd.dma_start(out=wk_sb[:], in_=wk)
            nc.sync.dma_start(out=x_sb[:, 0, 1], in_=xf[0, :, 1])
        else:
            nc.sync.dma_start(out=x_sb[:, 0], in_=xf[0])
            nc.gpsimd.dma_start(out=wk_sb[:], in_=wk)
        nc.sync.dma_start(out=x_sb[:, 1], in_=xf[1])
        nc.gpsimd.dma_start(out=wv_sb[:], in_=wv)
        for kc in range(2, 8):
            eng = nc.sync if kc % 2 == 0 else nc.gpsimd
            eng.dma_start(out=x_sb[:, kc], in_=xf[kc])
        ck_sb = consts.tile([128, 2, TH], BF16)
        nc.gpsimd.dma_start(out=ck_sb[:], in_=ck)
        for idx in range(8):
            wq_piece(idx, nc.sync)
        cq_sb = consts.tile([128, 2, TL], BF16)
        nc.gpsimd.dma_start(out=cq_sb[:], in_=cq)
        vb_sb = consts.tile([1, 640], BF16)
        nc.gpsimd.dma_start(out=vb_sb[:], in_=vb)
        wp_sb = persist.tile([128, 8, C], BF16, tag="wp")
        nc.sync.dma_start(out=wp_sb[:], in_=wp)

        ones_sb = consts.tile([1, 128], BF16)
        nc.vector.memset(ones_sb[:], 1.0)
        # identity for PE transpose
        ident = consts.tile([128, 128], BF16)
        nc.gpsimd.memset(ident[:], 1.0)
        nc.gpsimd.affine_select(out=ident[:], in_=ident[:],
                                compare_op=mybir.AluOpType.is_ge, fill=0.0,
                                base=0, channel_multiplier=1, pattern=[[-1, 128]])
        nc.gpsimd.affine_select(out=ident[:], in_=ident[:],
                                compare_op=mybir.AluOpType.is_ge, fill=0.0,
                                base=0, channel_multiplier=-1, pattern=[[1, 128]])
        # causal-edge multiplicative mask (keep key_p <= q_f), replicated x2
        maskC = consts.tile([128, 2, 128], BF16)
        nc.gpsimd.memset(maskC[:], 1.0)
        nc.gpsimd.affine_select(out=maskC[:], in_=maskC[:],
                                compare_op=mybir.AluOpType.is_ge, fill=0.0,
                                base=0, channel_multiplier=-1,
                                pattern=[[0, 2], [1, 128]])

        # persistent compute tensors
        qT = [persist.tile([64, TL], BF16, tag=f"qT{h}", name=f"qT{h}") for h in range(H)]
        kT = [persist.tile([64, TH], BF16, tag=f"kT{g}", name=f"kT{g}") for g in range(KV)]
        v65 = persist.tile([128, 10, 4 * 65], BF16, tag="v65")
        yv = persist.tile([128, 2, 8, 128], BF16, tag="yv")

        def contraction(out_ap, w_tile, wc0, wc1, n0, n1, x_cols_off, swap=False,
                        kc_range=range(8), kp_range=range(4), start_kc=0,
                        stop_kp=3):
            """Accumulate out += W[:, wc0:wc1].T @ X[:, n0+off:n1+off] over C.

            swap=True computes X.T @ W (x stationary) for the v projection.
            bf16: 8 matmuls; fp8: 8 DoubleRow (w(hi,lo) x x(hi,hi)) +
            4 DoubleRow (w(hi_k,hi_k+1) x x(lo_k,lo_k+1))."""
            if not USE_FP8:
                for kc in kc_range:
                    a = w_tile[:, kc, wc0:wc1]
                    b = x_sb[:, kc, x_cols_off + n0:x_cols_off + n1]
                    if swap:
                        a, b = b, a
                    nc.tensor.matmul(out_ap, a, b, start=(kc == start_kc),
                                     stop=(kc == 7))
                return
            def rep2(sl):
                return dataclasses.replace(
                    sl, ap=[sl.ap[0], [0, 2]] + list(sl.ap[1:]))
            for kc in kc_range:
                if swap:
                    # lhsT x (hi,lo) natural; rhs w (hi,hi) stride-0
                    a = x_sb[:, kc, 0:2, x_cols_off + n0:x_cols_off + n1]
                    b = rep2(w_tile[:, kc, 0, wc0:wc1])
                else:
                    # lhsT w (hi,lo) natural; rhs x (hi,hi) stride-0
                    a = w_tile[:, kc, 0:2, wc0:wc1]
                    b = rep2(x_sb[:, kc, 0, x_cols_off + n0:x_cols_off + n1])
                nc.tensor.matmul(out_ap, a, b, start=(kc == start_kc), stop=False,
                                 perf_mode=DR)
            for kp in kp_range:
                if swap:
                    # lhsT x-hi pairs; rhs w-lo pairs
                    a = x_sb[:, 2 * kp:2 * kp + 2, 0,
                             x_cols_off + n0:x_cols_off + n1]
                    b = w_tile[:, 2 * kp:2 * kp + 2, 1, wc0:wc1]
                else:
                    # lhsT w-hi pairs; rhs x-lo pairs
                    a = w_tile[:, 2 * kp:2 * kp + 2, 0, wc0:wc1]
                    b = x_sb[:, 2 * kp:2 * kp + 2, 1,
                             x_cols_off + n0:x_cols_off + n1]
                nc.tensor.matmul(out_ap, a, b, start=False, stop=(kp == stop_kp),
                                 perf_mode=DR)

        # ======== phase A: k-proj, v-proj, k-rope ========
        actx = contextlib.ExitStack()
        apool = actx.enter_context(tc.tile_pool(name="apool", bufs=1, space="PSUM"))
        vpool = actx.enter_context(tc.tile_pool(name="vpool", bufs=2, space="PSUM"))
        ropes = ctx.enter_context(tc.tile_pool(name="ropes", bufs=2))

        def rope_pair(pe, po, cs_sb, c0, c1, tag, bufs=2):
            n = c1 - c0
            e_sb = ropes.tile([128, n], BF16, tag=f"e{tag}", name="e_sb", bufs=bufs)
            o_sb = ropes.tile([128, n], BF16, tag=f"o{tag}", name="o_sb", bufs=bufs)
            nc.scalar.copy(e_sb[:], pe)
            nc.scalar.copy(o_sb[:], po)
            ne = ropes.tile([128, n], BF16, tag=f"r0{tag}", name="ne", bufs=bufs)
            no_ = ropes.tile([128, n], BF16, tag=f"r1{tag}", name="no_", bufs=bufs)
            t1 = ropes.tile([128, n], BF16, tag=f"r2{tag}", name="t1", bufs=bufs)
            t2 = ropes.tile([128, n], BF16, tag=f"r3{tag}", name="t2", bufs=bufs)
            nc.vector.tensor_mul(t1[:], e_sb[:], cs_sb[:, 0, c0:c1])
            nc.vector.tensor_mul(t2[:], o_sb[:], cs_sb[:, 1, c0:c1])
            nc.vector.tensor_sub(ne[:], t1[:], t2[:])
            nc.vector.tensor_mul(t1[:], e_sb[:], cs_sb[:, 1, c0:c1])
            nc.vector.tensor_mul(t2[:], o_sb[:], cs_sb[:, 0, c0:c1])
            nc.vector.tensor_add(no_[:], t1[:], t2[:])
            return ne, no_

        # k-proj + first v tiles emitted per-kc so PE tracks the x-slab DMAs
        kpe = apool.tile([128, TH], F32, tag="kpe")
        kpo = apool.tile([128, TH], F32, tag="kpo")
        pv0 = vpool.tile([128, 256], F32, tag="pv", name="pv")
        pv1 = vpool.tile([128, 256], F32, tag="pv", name="pv")
        spans = ((0, 512), (512, 1024), (1024, 1280))
        for kc in range(8):
            for (n0, n1) in spans:
                contraction(kpe[:, n0:n1], wk_sb, 0, 128, n0, n1, 0,
                            kc_range=(kc,), kp_range=())
                contraction(kpo[:, n0:n1], wk_sb, 128, 256, n0, n1, 0,
                            kc_range=(kc,), kp_range=())
            for tt, pv in ((0, pv0), (1, pv1)):
                contraction(pv[:], wv_sb, 0, 256, tt * 128, (tt + 1) * 128, 0,
                            swap=True, kc_range=(kc,), kp_range=())
            if USE_FP8 and kc % 2 == 1:
                kp = kc // 2
                for (n0, n1) in spans:
                    contraction(kpe[:, n0:n1], wk_sb, 0, 128, n0, n1, 0,
                                kc_range=(), kp_range=(kp,), start_kc=-1,
                                stop_kp=3)
                    contraction(kpo[:, n0:n1], wk_sb, 128, 256, n0, n1, 0,
                                kc_range=(), kp_range=(kp,), start_kc=-1,
                                stop_kp=3)

        def v_finish(tt, pv):
            v3 = v65[:, tt, :].rearrange("p (g c) -> p g c", c=65)
            if USE_FP8:
                nc.scalar.mul(v3[:, :, 0:64],
                              pv[:].rearrange("p (g c) -> p g c", c=64), DESCALE)
            else:
                nc.scalar.copy(v3[:, :, 0:64],
                               pv[:].rearrange("p (g c) -> p g c", c=64))
            nc.vector.memset(v3[:, :, 64:65], 1.0)

        for tt, pv in ((0, pv0), (1, pv1)):
            if USE_FP8:
                contraction(pv[:], wv_sb, 0, 256, tt * 128, (tt + 1) * 128, 0,
                            swap=True, kc_range=(), kp_range=range(4), start_kc=-1)
            v_finish(tt, pv)
        for tt in range(2, 10):
            pv = vpool.tile([128, 256], F32, tag="pv", name="pv")
            contraction(pv[:], wv_sb, 0, 256, tt * 128, (tt + 1) * 128, 0, swap=True)
            v_finish(tt, pv)

        kne, kno = rope_pair(kpe[:], kpo[:], ck_sb, 0, TH, "k", bufs=1)
        for g in range(KV):
            nc.gpsimd.dma_start(out=kT[g][0:32, :], in_=kne[g * 32:(g + 1) * 32, :])
            nc.gpsimd.dma_start(out=kT[g][32:64, :], in_=kno[g * 32:(g + 1) * 32, :])
        actx.close()

        # ======== phase B: q-proj + rope ========
        bctx = contextlib.ExitStack()
        qo = bctx.enter_context(tc.tile_pool(name="qo", bufs=2, space="PSUM"))
        for c4 in range(4):
            qpe = qo.tile([128, TL], F32, tag="qpe", name="qpe")
            qpo = qo.tile([128, TL], F32, tag="qpo", name="qpo")
            for th in range(2):
                t0, t1 = th * 512, (th + 1) * 512
                contraction(qpe[:, t0:t1], wq_sb, c4 * 128, (c4 + 1) * 128,
                            t0, t1, WIN)
                contraction(qpo[:, t0:t1], wq_sb,
                            512 + c4 * 128, 512 + (c4 + 1) * 128, t0, t1, WIN)
            ne, no_ = rope_pair(qpe[:], qpo[:], cq_sb, 0, TL, "q")
            for j in range(4):
                h = c4 * 4 + j
                enge = nc.sync if j % 2 == 0 else nc.gpsimd
                engo = nc.gpsimd if j % 2 == 0 else nc.sync
                enge.dma_start(out=qT[h][0:32, :],
                               in_=ne[j * 32:(j + 1) * 32, :])
                engo.dma_start(out=qT[h][32:64, :],
                               in_=no_[j * 32:(j + 1) * 32, :])

        # ======== phase C: attention pipeline ========
        bctx.close()
        stp = ctx.enter_context(tc.tile_pool(name="stp", bufs=3, space="PSUM"))
        opool = ctx.enter_context(tc.tile_pool(name="opool", bufs=2, space="PSUM"))
        pts = ctx.enter_context(tc.tile_pool(name="pts", bufs=6))
        ysb = ctx.enter_context(tc.tile_pool(name="ysb", bufs=6))
        osb = ctx.enter_context(tc.tile_pool(name="osb", bufs=2))

        NSLOT = 64
        st_t = {}
        pt_t = {}
        y_t = {}

        def get_st(s):
            if s not in st_t:
                st_t[s] = stp.tile([128, 962], F32, tag="st", name="st")
            return st_t[s]

        def emit_scores(s):
            qb, P = divmod(s, 8)
            g, jj = divmod(P, 2)
            st = get_st(s)
            for ji in range(2):
                h = 4 * g + 2 * jj + ji
                for cc in range(3):
                    o = st[:, ji * 384 + cc * 128: ji * 384 + (cc + 1) * 128]
                    has_vb = (qb + cc) <= 1
                    nc.tensor.matmul(
                        o, kT[g][:, (qb + cc) * 128:(qb + cc + 1) * 128],
                        qT[h][:, qb * 128:(qb + 1) * 128],
                        start=True, stop=not has_vb)
                    if has_vb:
                        nc.tensor.matmul(
                            o, vb_sb[:, (qb + cc) * 128:(qb + cc + 1) * 128],
                            ones_sb[:, 0:128], start=False, stop=True)
            pt = pts.tile([128, 768], BF16, tag="pt", name="pt")
            pt_t[s] = pt
            nc.scalar.activation(pt[:], st[:, 0:768],
                                 mybir.ActivationFunctionType.Exp, scale=0.125)
            # lower band edge (keep key_p > q_f) on gpsimd
            lo_edge = dataclasses.replace(
                pt[:, 0:128], ap=[pt.ap[0], [384, 2], [1, 128]])
            nc.gpsimd.affine_select(out=lo_edge, in_=lo_edge,
                                    compare_op=mybir.AluOpType.is_ge, fill=0.0,
                                    base=-1, channel_multiplier=1,
                                    pattern=[[0, 2], [-1, 128]])
            # causal edge (keep key_p <= q_f) on DVE
            hi_edge = dataclasses.replace(
                pt[:, 256:384], ap=[pt.ap[0], [384, 2], [1, 128]])
            nc.vector.tensor_mul(hi_edge, hi_edge, maskC[:])

        def emit_av(s):
            qb, P = divmod(s, 8)
            g = P // 2
            pt = pt_t[s]
            stq = get_st(s + 2)
            for ji in range(2):
                o = stq[:, 768 + ji * 65: 833 + ji * 65]
                for cc in range(3):
                    nc.tensor.matmul(
                        o, pt[:, ji * 384 + cc * 128: ji * 384 + (cc + 1) * 128],
                        v65[:, qb + cc, g * 65:(g + 1) * 65],
                        start=(cc == 0), stop=(cc == 2))
            # reciprocal of the two denominators, then one batched normalize
            # (per-head scale broadcast along the 64 channels via stride-0)
            rq = ysb.tile([128, 2], F32, tag="rq", name="rq")
            den = dataclasses.replace(
                stq[:, 832:833], ap=[stq.ap[0], [65, 2]])
            nc.vector.reciprocal(rq[:], den)
            y = ysb.tile([128, 128], BF16, tag="y", name="y")
            y_t[s] = y
            yqv = dataclasses.replace(
                stq[:, 768:832], ap=[stq.ap[0], [65, 2], [1, 64]])
            rqv = dataclasses.replace(
                rq[:, 0:1], ap=[rq.ap[0], [1, 2], [0, 64]])
            nc.vector.tensor_mul(
                y[:].rearrange("p (a b) -> p a b", b=64), yqv, rqv)

        def emit_tr(s):
            qb, P = divmod(s, 8)
            stq = get_st(s + 3)
            yt = stq[:, 898:962].bitcast(BF16)
            nc.tensor.transpose(yt, y_t[s][:], ident[:])
            nc.vector.tensor_copy(out=yv[:, qb % 2, P, :], in_=yt)

        op_t = {}
        o_t = {}

        def emit_oproj_part(qb, i):
            if i == 0:
                op_t[(qb, 0)] = opool.tile([128, 512], F32, tag="op0",
                                           name="p0", bufs=1)
                op_t[(qb, 1)] = opool.tile([128, 512], F32, tag="op1",
                                           name="p1", bufs=1)
            for half in (0, 1):
                pp = op_t[(qb, half)]
                for pr in (2 * i, 2 * i + 1):
                    nc.tensor.matmul(
                        pp[:], yv[:, qb % 2, pr, :],
                        wp_sb[:, pr, half * 512:(half + 1) * 512],
                        start=(pr == 0), stop=(pr == 7))

        def emit_osb(qb):
            o_sb = osb.tile([128, C], F32, tag="o_sb", name="o_sb")
            nc.scalar.copy(o_sb[:, 0:512], op_t[(qb, 0)][:])
            eng = nc.sync if qb % 2 == 0 else nc.gpsimd
            eng.dma_start(out=out[qb * 128:(qb + 1) * 128, 0:512],
                          in_=o_sb[:, 0:512])
            nc.vector.tensor_copy(out=o_sb[:, 512:1024], in_=op_t[(qb, 1)][:])
            eng2 = nc.gpsimd if qb % 2 == 0 else nc.sync
            eng2.dma_start(out=out[qb * 128:(qb + 1) * 128, 512:1024],
                           in_=o_sb[:, 512:1024])

        for s in range(NSLOT + 12):
            if 3 <= s and (s - 3) < NSLOT:
                emit_av(s - 3)
            if 5 <= s and (s - 5) < NSLOT:
                emit_tr(s - 5)
            if s < NSLOT:
                emit_scores(s)
            for i in range(4):
                if s - 14 - i >= 0 and (s - 14 - i) % 8 == 0:
                    qb = (s - 14 - i) // 8
                    if qb < 8:
                        emit_oproj_part(qb, i)
            if s >= 19 and (s - 19) % 8 == 0 and (s - 19) // 8 < 8:
                emit_osb((s - 19) // 8)


_PROGRAM_CACHE = {}


def _get_program():
    if "nc" not in _PROGRAM_CACHE:
        _PROGRAM_CACHE["nc"] = _build_program()
    return _PROGRAM_CACHE["nc"]


def _f8_hilo(a):
    """Quantize to fp8 e4m3 hi + lo residual. a: float32 ndarray."""
    hi = a.astype(F8)
    lo = (a - hi.astype(np.float32)).astype(F8)
    return hi, lo


def prepare_in_maps(x, freqs_cos, freqs_sin, w_attn, b_attn, w_proj, b_proj):
    x = np.asarray(x, dtype=np.float32)
    freqs_cos = np.asarray(freqs_cos, dtype=np.float32)
    freqs_sin = np.asarray(freqs_sin, dtype=np.float32)
    w_attn = np.asarray(w_attn, dtype=np.float32)
    b_attn = np.asarray(b_attn, dtype=np.float32)
    w_proj = np.asarray(w_proj, dtype=np.float32)
    assert not np.any(b_attn), "kernel assumes zero qkv bias"

    # q/k channel permutation: evens block then odds block, head-major
    qch = np.arange(H * HD).reshape(H, 32, 2)
    q_perm = np.concatenate([qch[:, :, 0].reshape(-1), qch[:, :, 1].reshape(-1)])
    kch = H * HD + np.arange(KV * HD).reshape(KV, 32, 2)
    k_perm = np.concatenate([kch[:, :, 0].reshape(-1), kch[:, :, 1].reshape(-1)])
    wq_f = np.ascontiguousarray(w_attn[q_perm].T)              # (1024, 1024)
    wk_f = np.ascontiguousarray(w_attn[k_perm].T)              # (1024, 256)
    wv_f = np.ascontiguousarray(w_attn[(H + KV) * HD:].T)      # (1024, 256)
    wp_h = np.ascontiguousarray(
        w_proj.T.reshape(8, 128, C).transpose(1, 0, 2)).astype(BF)

    if USE_FP8:
        def wpack(w, width):
            hi, lo = _f8_hilo(w.reshape(8, 128, width) * SW)
            # (k, p, 2, width) -> (p, k, 2, width)
            return np.ascontiguousarray(
                np.stack([hi, lo], axis=2).transpose(1, 0, 2, 3))
        wq_h = wpack(wq_f, 1024)
        wk_h = wpack(wk_f, 256)
        wv_h = wpack(wv_f, 256)
        tab_scale = DESCALE
    else:
        wq_h = np.ascontiguousarray(
            wq_f.reshape(8, 128, 1024).transpose(1, 0, 2)).astype(BF)
        wk_h = np.ascontiguousarray(
            wk_f.reshape(8, 128, 256).transpose(1, 0, 2)).astype(BF)
        wv_h = np.ascontiguousarray(
            wv_f.reshape(8, 128, 256).transpose(1, 0, 2)).astype(BF)
        tab_scale = 1.0

    cos4 = np.tile(freqs_cos.T, (4, 1)).astype(np.float32) * tab_scale  # (128, T)
    sin4 = np.tile(freqs_sin.T, (4, 1)).astype(np.float32) * tab_scale

    in_maps = []
    for core in range(8):
        b, h = divmod(core, 2)
        t0 = h * TL
        xs = np.zeros((TH, C), dtype=np.float32)
        lo_ = max(0, t0 - WIN)
        xs[TH - (t0 + TL - lo_):] = x[b, lo_:t0 + TL]
        if USE_FP8:
            xk = xs.T.reshape(8, 128, TH) * SX
            x_hi, x_lo = _f8_hilo(xk)
            xh = np.ascontiguousarray(np.stack([x_hi, x_lo], axis=2))
        else:
            xh = np.ascontiguousarray(xs.T.reshape(8, 128, TH)).astype(BF)
        vbv = np.zeros((1, 640), dtype=np.float32)
        if h == 0:
            vbv[0, :WIN] = NEG
        cpad = np.zeros((128, TH), dtype=np.float32)
        spad = np.zeros((128, TH), dtype=np.float32)
        cpad[:, TH - (t0 + TL - lo_):] = cos4[:, lo_:t0 + TL]
        spad[:, TH - (t0 + TL - lo_):] = sin4[:, lo_:t0 + TL]
        in_maps.append({
            "xf": xh,
            "wq": wq_h, "wk": wk_h, "wv": wv_h, "wp": wp_h,
            "cq": np.ascontiguousarray(
                np.stack([cos4[:, t0:t0 + TL],
                          sin4[:, t0:t0 + TL]], axis=1)).astype(BF),
            "ck": np.ascontiguousarray(
                np.stack([cpad, spad], axis=1)).astype(BF),
            "vb": vbv.astype(BF),
        })

    return in_maps


def kernel(**inputs):
    in_maps = prepare_in_maps(**inputs)
    nc = _get_program()
    res = run_bass_kernel_spmd(nc, in_maps, list(range(8)))
    return _gather(res, np.asarray(inputs["b_proj"], dtype=np.float32))


def _gather(res, b_proj):
    out = np.empty((B, T, C), dtype=np.float32)
    for core in range(8):
        b, h = divmod(core, 2)
        out[b, h * TL:(h + 1) * TL] = res.results[core]["out"]
    if np.any(b_proj):
        out += b_proj
    return out


# revision 76
# speedup vs baseline: 1.0118x; 1.0118x over previous
"""Sliding-window GQA causal self-attention block for 8 trn2 NeuronCores.

Sharding: batch (4) x T-halves (2) -> 8 cores, no collectives. Each core gets
x.T for its T-half plus a 256-row key/value halo and computes its (1024, 1024)
slice of the output.

Schedule (single continuous PE stream, cost-model: matmul = out-free-size
rows):
  A: k-proj then v-proj (fp8 hi/lo DoubleRow: 6N rows per C-contraction vs
     8N bf16), k-rope + kT SBUF-DMAs overlapped with v-proj.
  B: q-proj per (t-half, c4) in [128,512] psum chunks, rope, qT DMAs.
  C: 64-slot pipeline over (qb, head-pair): scores (keys on partitions,
     3x128 cols) -> exp on ACT (2 heads batched, scale=1/8, no max-sub) ->
     band-edge masks (gpsimd affine_select for the lower edge, DVE mul for
     the causal edge) -> att@v with queries on psum partitions (N=65/call,
     ones column gives softmax denominators, 3-slot lag) -> reciprocal +
     one stride-0-broadcast normalize mul on DVE -> PE transpose (identity
     matmul) into spare bytes of a st tile -> output projection spread 4
     matmuls/slot into dedicated psum banks -> psum->sbuf on ACT+DVE, DMA
     out (split across sync/gpsimd queues).
PSUM: st [128,962]f32 x3 bufs = 6 banks (cols 0:768 scores, 768:898 att@v
accumulators written two slots ahead, 898:962 bf16 transpose target) +
op0/op1 [128,512]f32 = 2 banks. GPSIMD never touches PSUM (hw illegal);
DMA never touches PSUM (bass asserts).
"""

import dataclasses

import numpy as np
import ml_dtypes

import concourse.bass as bass
import concourse.mybir as mybir
import concourse.tile as tile
from concourse import bacc
from concourse.bass_utils import run_bass_kernel_spmd

BF = ml_dtypes.bfloat16
F8 = ml_dtypes.float8_e4m3
F32 = mybir.dt.float32
BF16 = mybir.dt.bfloat16
FP8 = mybir.dt.float8e4

B, T, C = 4, 2048, 1024
H, KV, HD = 16, 4, 64
WIN = 256
TL = T // 2            # 1024 own rows per core
TH = TL + WIN          # 1280 with halo
NEG = -30000.0

USE_FP8 = True
SX = 4.0               # fp8 scale for x
SW = 64.0              # fp8 scale for w_attn (qk and v slices)
DESCALE = 1.0 / (SX * SW)

DR = mybir.MatmulPerfMode.DoubleRow


def _build_program():
    nc = bacc.Bacc("TRN2", target_bir_lowering=False, debug=False, num_devices=8)
    dt = mybir.dt
    if USE_FP8:
        # x: 3-slot (hi,hi,lo); weights: 2-slot (hi,lo) along the pair dim
        xf = nc.dram_tensor("xf", [8, 128, 2, TH], dt.float8e4, kind="ExternalInput").ap()
        wq = nc.dram_tensor("wq", [128, 8, 2, 1024], dt.float8e4, kind="ExternalInput").ap()
        wk = nc.dram_tensor("wk", [128, 8, 2, 256], dt.float8e4, kind="ExternalInput").ap()
        wv = nc.dram_tensor("wv", [128, 8, 2, 256], dt.float8e4, kind="ExternalInput").ap()
    else:
        xf = nc.dram_tensor("xf", [8, 128, TH], dt.bfloat16, kind="ExternalInput").ap()
        wq = nc.dram_tensor("wq", [128, 8, 1024], dt.bfloat16, kind="ExternalInput").ap()
        wk = nc.dram_tensor("wk", [128, 8, 256], dt.bfloat16, kind="ExternalInput").ap()
        wv = nc.dram_tensor("wv", [128, 8, 256], dt.bfloat16, kind="ExternalInput").ap()
    wp = nc.dram_tensor("wp", [128, 8, C], dt.bfloat16, kind="ExternalInput").ap()
    cq = nc.dram_tensor("cq", [128, 2, TL], dt.bfloat16, kind="ExternalInput").ap()
    ck = nc.dram_tensor("ck", [128, 2, TH], dt.bfloat16, kind="ExternalInput").ap()
    vb = nc.dram_tensor("vb", [1, 640], dt.bfloat16, kind="ExternalInput").ap()
    out = nc.dram_tensor("out", [TL, C], dt.float32, kind="ExternalOutput").ap()

    with tile.TileContext(nc) as tc:
        _kernel_body(tc, nc, xf, wq, wk, wv, wp, cq, ck, vb, out)
    nc.compile()
    return nc


def _kernel_body(tc, nc, xf, wq, wk, wv, wp, cq, ck, vb, out):
    import contextlib
    ctx = contextlib.ExitStack()
    with ctx:
        consts = ctx.enter_context(tc.tile_pool(name="consts", bufs=1))
        persist = ctx.enter_context(tc.tile_pool(name="persist", bufs=1))

        # ---- persistent inputs (DMA order = consumption order) ----
        if USE_FP8:
            x_sb = persist.tile([128, 8, 2, TH], FP8, tag="x")
            wq_sb = persist.tile([128, 8, 2, 1024], FP8, tag="wq")
            wk_sb = persist.tile([128, 8, 2, 256], FP8, tag="wk")
            wv_sb = persist.tile([128, 8, 2, 256], FP8, tag="wv")
        else:
            x_sb = persist.tile([128, 8, TH], BF16, tag="x")
            wq_sb = persist.tile([128, 8, 1024], BF16, tag="wq")
            wk_sb = persist.tile([128, 8, 256], BF16, tag="wk")
            wv_sb = persist.tile([128, 8, 256], BF16, tag="wv")
        def wq_piece(idx, eng):
            c4, half = divmod(idx, 2)
            c0 = half * 512 + c4 * 128
            if USE_FP8:
                eng.dma_start(out=wq_sb[:, :, :, c0:c0 + 128],
                              in_=wq[:, :, :, c0:c0 + 128])
            else:
                eng.dma_start(out=wq_sb[:, :, c0:c0 + 128],
                              in_=wq[:, :, c0:c0 + 128])

        if USE_FP8:
            nc.sync.dma_start(out=x_sb[:, 0, 0], in_=xf[0, :, 0])
            nc.gpsimd.dma_start(out=wk_sb[:], in_=wk)
            nc.sync.dma_start(out=x_sb[:, 0, 1], in_=xf[0, :, 1])
        else:
            nc.sync.dma_start(out=x_sb[:, 0], in_=xf[0])
            nc.gpsimd.dma_start(out=wk_sb[:], in_=wk)
        nc.sync.dma_start(out=x_sb[:, 1], in_=xf[1])
        nc.gpsimd.dma_start(out=wv_sb[:], in_=wv)
        for kc in range(2, 8):
            eng = nc.sync if kc % 2 == 0 else nc.gpsimd
            eng.dma_start(out=x_sb[:, kc], in_=xf[kc])
        ck_sb = consts.tile([128, 2, TH], BF16)
        nc.gpsimd.dma_start(out=ck_sb[:], in_=ck)
        for idx in range(8):
            wq_piece(idx, nc.sync)
        cq_sb = consts.tile([128, 2, TL], BF16)
        nc.gpsimd.dma_start(out=cq_sb[:], in_=cq)
        vb_sb = consts.tile([1, 640], BF16)
        nc.gpsimd.dma_start(out=vb_sb[:], in_=vb)
        wp_sb = persist.tile([128, 8, C], BF16, tag="wp")
        nc.sync.dma_start(out=wp_sb[:], in_=wp)

        ones_sb = consts.tile([1, 128], BF16)
        nc.vector.memset(ones_sb[:], 1.0)
        # identity for PE transpose
        ident = consts.tile([128, 128], BF16)
        nc.gpsimd.memset(ident[:], 1.0)
        nc.gpsimd.affine_select(out=ident[:], in_=ident[:],
                                compare_op=mybir.AluOpType.is_ge, fill=0.0,
                                base=0, channel_multiplier=1, pattern=[[-1, 128]])
        nc.gpsimd.affine_select(out=ident[:], in_=ident[:],
                                compare_op=mybir.AluOpType.is_ge, fill=0.0,
                                base=0, channel_multiplier=-1, pattern=[[1, 128]])
        # band-edge masks: plane 0 keeps key_p > q_f, plane 1 keeps key_p <= q_f
        maskC = consts.tile([128, 2, 128], BF16)
        nc.gpsimd.memset(maskC[:], 1.0)
        nc.gpsimd.affine_select(out=maskC[:, 0, :], in_=maskC[:, 0, :],
                                compare_op=mybir.AluOpType.is_ge, fill=0.0,
                                base=-1, channel_multiplier=1,
                                pattern=[[-1, 128]])
        nc.gpsimd.affine_select(out=maskC[:, 1, :], in_=maskC[:, 1, :],
                                compare_op=mybir.AluOpType.is_ge, fill=0.0,
                                base=0, channel_multiplier=-1,
                                pattern=[[1, 128]])

        # persistent compute tensors
        qT = [persist.tile([64, TL], BF16, tag=f"qT{h}", name=f"qT{h}") for h in range(H)]
        kT = [persist.tile([64, TH], BF16, tag=f"kT{g}", name=f"kT{g}") for g in range(KV)]
        v65 = persist.tile([128, 10, 4 * 65], BF16, tag="v65")
        yv = persist.tile([128, 2, 8, 128], BF16, tag="yv")

        def contraction(out_ap, w_tile, wc0, wc1, n0, n1, x_cols_off, swap=False,
                        kc_range=range(8), kp_range=range(4), start_kc=0,
                        stop_kp=3):
            """Accumulate out += W[:, wc0:wc1].T @ X[:, n0+off:n1+off] over C.

            swap=True computes X.T @ W (x stationary) for the v projection.
            bf16: 8 matmuls; fp8: 8 DoubleRow (w(hi,lo) x x(hi,hi)) +
            4 DoubleRow (w(hi_k,hi_k+1) x x(lo_k,lo_k+1))."""
            if not USE_FP8:
                for kc in kc_range:
                    a = w_tile[:, kc, wc0:wc1]
                    b = x_sb[:, kc, x_cols_off + n0:x_cols_off + n1]
                    if swap:
                        a, b = b, a
                    nc.tensor.matmul(out_ap, a, b, start=(kc == start_kc),
                                     stop=(kc == 7))
                return
            def rep2(sl):
                return dataclasses.replace(
                    sl, ap=[sl.ap[0], [0, 2]] + list(sl.ap[1:]))
            for kc in kc_range:
                if swap:
                    # lhsT x (hi,lo) natural; rhs w (hi,hi) stride-0
                    a = x_sb[:, kc, 0:2, x_cols_off + n0:x_cols_off + n1]
                    b = rep2(w_tile[:, kc, 0, wc0:wc1])
                else:
                    # lhsT w (hi,lo) natural; rhs x (hi,hi) stride-0
                    a = w_tile[:, kc, 0:2, wc0:wc1]
                    b = rep2(x_sb[:, kc, 0, x_cols_off + n0:x_cols_off + n1])
                nc.tensor.matmul(out_ap, a, b, start=(kc == start_kc), stop=False,
                                 perf_mode=DR)
            for kp in kp_range:
                if swap:
                    # lhsT x-hi pairs; rhs w-lo pairs
                    a = x_sb[:, 2 * kp:2 * kp + 2, 0,
                             x_cols_off + n0:x_cols_off + n1]
                    b = w_tile[:, 2 * kp:2 * kp + 2, 1, wc0:wc1]
                else:
                    # lhsT w-hi pairs; rhs x-lo pairs
                    a = w_tile[:, 2 * kp:2 * kp + 2, 0, wc0:wc1]
                    b = x_sb[:, 2 * kp:2 * kp + 2, 1,
                             x_cols_off + n0:x_cols_off + n1]
                nc.tensor.matmul(out_ap, a, b, start=False, stop=(kp == stop_kp),
                                 perf_mode=DR)

        # ======== phase A: k-proj, v-proj, k-rope ========
        actx = contextlib.ExitStack()
        apool = actx.enter_context(tc.tile_pool(name="apool", bufs=1, space="PSUM"))
        vpool = actx.enter_context(tc.tile_pool(name="vpool", bufs=2, space="PSUM"))
        ropes = ctx.enter_context(tc.tile_pool(name="ropes", bufs=2))

        def rope_pair(pe, po, cs_sb, c0, c1, tag, bufs=2):
            n = c1 - c0
            e_sb = ropes.tile([128, n], BF16, tag=f"e{tag}", name="e_sb", bufs=bufs)
            o_sb = ropes.tile([128, n], BF16, tag=f"o{tag}", name="o_sb", bufs=bufs)
            nc.scalar.copy(e_sb[:], pe)
            nc.scalar.copy(o_sb[:], po)
            ne = ropes.tile([128, n], BF16, tag=f"r0{tag}", name="ne", bufs=bufs)
            no_ = ropes.tile([128, n], BF16, tag=f"r1{tag}", name="no_", bufs=bufs)
            t1 = ropes.tile([128, n], BF16, tag=f"r2{tag}", name="t1", bufs=bufs)
            t2 = ropes.tile([128, n], BF16, tag=f"r3{tag}", name="t2", bufs=bufs)
            nc.vector.tensor_mul(t1[:], e_sb[:], cs_sb[:, 0, c0:c1])
            nc.vector.tensor_mul(t2[:], o_sb[:], cs_sb[:, 1, c0:c1])
            nc.vector.tensor_sub(ne[:], t1[:], t2[:])
            nc.vector.tensor_mul(t1[:], e_sb[:], cs_sb[:, 1, c0:c1])
            nc.vector.tensor_mul(t2[:], o_sb[:], cs_sb[:, 0, c0:c1])
            nc.vector.tensor_add(no_[:], t1[:], t2[:])
            return ne, no_

        # k-proj + first v tiles emitted per-kc so PE tracks the x-slab DMAs
        kpe = apool.tile([128, TH], F32, tag="kpe")
        kpo = apool.tile([128, TH], F32, tag="kpo")
        pv0 = vpool.tile([128, 256], F32, tag="pv", name="pv")
        pv1 = vpool.tile([128, 256], F32, tag="pv", name="pv")
        spans = ((0, 512), (512, 1024), (1024, 1280))
        for kc in range(8):
            for (n0, n1) in spans:
                contraction(kpe[:, n0:n1], wk_sb, 0, 128, n0, n1, 0,
                            kc_range=(kc,), kp_range=())
                contraction(kpo[:, n0:n1], wk_sb, 128, 256, n0, n1, 0,
                            kc_range=(kc,), kp_range=())
            for tt, pv in ((0, pv0), (1, pv1)):
                contraction(pv[:], wv_sb, 0, 256, tt * 128, (tt + 1) * 128, 0,
                            swap=True, kc_range=(kc,), kp_range=())
            if USE_FP8 and kc % 2 == 1:
                kp = kc // 2
                for (n0, n1) in spans:
                    contraction(kpe[:, n0:n1], wk_sb, 0, 128, n0, n1, 0,
                                kc_range=(), kp_range=(kp,), start_kc=-1,
                                stop_kp=3)
                    contraction(kpo[:, n0:n1], wk_sb, 128, 256, n0, n1, 0,
                                kc_range=(), kp_range=(kp,), start_kc=-1,
                                stop_kp=3)

        def v_finish(tt, pv):
            v3 = v65[:, tt, :].rearrange("p (g c) -> p g c", c=65)
            if USE_FP8:
                nc.scalar.mul(v3[:, :, 0:64],
                              pv[:].rearrange("p (g c) -> p g c", c=64), DESCALE)
            else:
                nc.scalar.copy(v3[:, :, 0:64],
                               pv[:].rearrange("p (g c) -> p g c", c=64))
            nc.vector.memset(v3[:, :, 64:65], 1.0)

        for tt, pv in ((0, pv0), (1, pv1)):
            if USE_FP8:
                contraction(pv[:], wv_sb, 0, 256, tt * 128, (tt + 1) * 128, 0,
                            swap=True, kc_range=(), kp_range=range(4), start_kc=-1)
            v_finish(tt, pv)
        for tt in range(2, 10):
            pv = vpool.tile([128, 256], F32, tag="pv", name="pv")
            contraction(pv[:], wv_sb, 0, 256, tt * 128, (tt + 1) * 128, 0, swap=True)
            v_finish(tt, pv)

        kne, kno = rope_pair(kpe[:], kpo[:], ck_sb, 0, TH, "k", bufs=1)
        for g in range(KV):
            nc.gpsimd.dma_start(out=kT[g][0:32, :], in_=kne[g * 32:(g + 1) * 32, :])
            nc.gpsimd.dma_start(out=kT[g][32:64, :], in_=kno[g * 32:(g + 1) * 32, :])
        actx.close()

        # ======== phase B: q-proj + rope ========
        bctx = contextlib.ExitStack()
        qo = bctx.enter_context(tc.tile_pool(name="qo", bufs=2, space="PSUM"))
        for c4 in range(4):
            qpe = qo.tile([128, TL], F32, tag="qpe", name="qpe")
            qpo = qo.tile([128, TL], F32, tag="qpo", name="qpo")
            for th in range(2):
                t0, t1 = th * 512, (th + 1) * 512
                contraction(qpe[:, t0:t1], wq_sb, c4 * 128, (c4 + 1) * 128,
                            t0, t1, WIN)
                contraction(qpo[:, t0:t1], wq_sb,
                            512 + c4 * 128, 512 + (c4 + 1) * 128, t0, t1, WIN)
            ne, no_ = rope_pair(qpe[:], qpo[:], cq_sb, 0, TL, "q")
            for j in range(4):
                h = c4 * 4 + j
                enge = nc.sync if j % 2 == 0 else nc.gpsimd
                engo = nc.gpsimd if j % 2 == 0 else nc.sync
                enge.dma_start(out=qT[h][0:32, :],
                               in_=ne[j * 32:(j + 1) * 32, :])
                engo.dma_start(out=qT[h][32:64, :],
                               in_=no_[j * 32:(j + 1) * 32, :])

        # ======== phase C: attention pipeline ========
        bctx.close()
        stp = ctx.enter_context(tc.tile_pool(name="stp", bufs=3, space="PSUM"))
        opool = ctx.enter_context(tc.tile_pool(name="opool", bufs=2, space="PSUM"))
        pts = ctx.enter_context(tc.tile_pool(name="pts", bufs=6))
        ysb = ctx.enter_context(tc.tile_pool(name="ysb", bufs=6))
        osb = ctx.enter_context(tc.tile_pool(name="osb", bufs=2))

        NSLOT = 64
        st_t = {}
        pt_t = {}
        y_t = {}

        def get_st(s):
            if s not in st_t:
                st_t[s] = stp.tile([128, 962], F32, tag="st", name="st")
            return st_t[s]

        def emit_scores(s):
            qb, P = divmod(s, 8)
            g, jj = divmod(P, 2)
            st = get_st(s)
            for ji in range(2):
                h = 4 * g + 2 * jj + ji
                for cc in range(3):
                    o = st[:, ji * 384 + cc * 128: ji * 384 + (cc + 1) * 128]
                    has_vb = (qb + cc) <= 1
                    nc.tensor.matmul(
                        o, kT[g][:, (qb + cc) * 128:(qb + cc + 1) * 128],
                        qT[h][:, qb * 128:(qb + 1) * 128],
                        start=True, stop=not has_vb)
                    if has_vb:
                        nc.tensor.matmul(
                            o, vb_sb[:, (qb + cc) * 128:(qb + cc + 1) * 128],
                            ones_sb[:, 0:128], start=False, stop=True)
            pt = pts.tile([128, 768], BF16, tag="pt", name="pt")
            pt_t[s] = pt
            nc.scalar.activation(pt[:], st[:, 0:768],
                                 mybir.ActivationFunctionType.Exp, scale=0.125)
            # both band edges (4 blocks) in one DVE mul; mask planes
            # broadcast over the head dim via a stride-0 leading free dim
            edge4 = dataclasses.replace(
                pt[:, 0:128], ap=[pt.ap[0], [384, 2], [256, 2], [1, 128]])
            maskv = dataclasses.replace(
                maskC[:, 0, :], ap=[maskC.ap[0], [0, 2], [128, 2], [1, 128]])
            nc.vector.tensor_mul(edge4, edge4, maskv)

        def emit_av(s):
            qb, P = divmod(s, 8)
            g = P // 2
            pt = pt_t[s]
            stq = get_st(s + 2)
            for ji in range(2):
                o = stq[:, 768 + ji * 65: 833 + ji * 65]
                for cc in range(3):
                    nc.tensor.matmul(
                        o, pt[:, ji * 384 + cc * 128: ji * 384 + (cc + 1) * 128],
                        v65[:, qb + cc, g * 65:(g + 1) * 65],
                        start=(cc == 0), stop=(cc == 2))
            # reciprocal of the two denominators, then one batched normalize
            # (per-head scale broadcast along the 64 channels via stride-0)
            rq = ysb.tile([128, 2], F32, tag="rq", name="rq")
            den = dataclasses.replace(
                stq[:, 832:833], ap=[stq.ap[0], [65, 2]])
            nc.vector.reciprocal(rq[:], den)
            y = ysb.tile([128, 128], BF16, tag="y", name="y")
            y_t[s] = y
            yqv = dataclasses.replace(
                stq[:, 768:832], ap=[stq.ap[0], [65, 2], [1, 64]])
            rqv = dataclasses.replace(
                rq[:, 0:1], ap=[rq.ap[0], [1, 2], [0, 64]])
            nc.vector.tensor_mul(
                y[:].rearrange("p (a b) -> p a b", b=64), yqv, rqv)

        def emit_tr(s):
            qb, P = divmod(s, 8)
            stq = get_st(s + 3)
            yt = stq[:, 898:962].bitcast(BF16)
            nc.tensor.transpose(yt, y_t[s][:], ident[:])
            nc.vector.tensor_copy(out=yv[:, qb % 2, P, :], in_=yt)

        op_t = {}
        o_t = {}

        def emit_oproj_part(qb, i):
            if i == 0:
                op_t[(qb, 0)] = opool.tile([128, 512], F32, tag="op0",
                                           name="p0", bufs=1)
                op_t[(qb, 1)] = opool.tile([128, 512], F32, tag="op1",
                                           name="p1", bufs=1)
            for half in (0, 1):
                pp = op_t[(qb, half)]
                for pr in (2 * i, 2 * i + 1):
                    nc.tensor.matmul(
                        pp[:], yv[:, qb % 2, pr, :],
                        wp_sb[:, pr, half * 512:(half + 1) * 512],
                        start=(pr == 0), stop=(pr == 7))

        def emit_osb(qb):
            o_sb = osb.tile([128, C], F32, tag="o_sb", name="o_sb")
            nc.scalar.copy(o_sb[:, 0:512], op_t[(qb, 0)][:])
            eng = nc.sync if qb % 2 == 0 else nc.gpsimd
            eng.dma_start(out=out[qb * 128:(qb + 1) * 128, 0:512],
                          in_=o_sb[:, 0:512])
            nc.vector.tensor_copy(out=o_sb[:, 512:1024], in_=op_t[(qb, 1)][:])
            eng2 = nc.gpsimd if qb % 2 == 0 else nc.sync
            eng2.dma_start(out=out[qb * 128:(qb + 1) * 128, 512:1024],
                           in_=o_sb[:, 512:1024])

        for s in range(NSLOT + 12):
            if 3 <= s and (s - 3) < NSLOT:
                emit_av(s - 3)
            if 5 <= s and (s - 5) < NSLOT:
                emit_tr(s - 5)
            if s < NSLOT:
                emit_scores(s)
            for i in range(4):
                if s - 14 - i >= 0 and (s - 14 - i) % 8 == 0:
                    qb = (s - 14 - i) // 8
                    if qb < 8:
                        emit_oproj_part(qb, i)
            if s >= 19 and (s - 19) % 8 == 0 and (s - 19) // 8 < 8:
                emit_osb((s - 19) // 8)


_PROGRAM_CACHE = {}


def _get_program():
    if "nc" not in _PROGRAM_CACHE:
        _PROGRAM_CACHE["nc"] = _build_program()
    return _PROGRAM_CACHE["nc"]


def _f8_hilo(a):
    """Quantize to fp8 e4m3 hi + lo residual. a: float32 ndarray."""
    hi = a.astype(F8)
    lo = (a - hi.astype(np.float32)).astype(F8)
    return hi, lo


def prepare_in_maps(x, freqs_cos, freqs_sin, w_attn, b_attn, w_proj, b_proj):
    x = np.asarray(x, dtype=np.float32)
    freqs_cos = np.asarray(freqs_cos, dtype=np.float32)
    freqs_sin = np.asarray(freqs_sin, dtype=np.float32)
    w_attn = np.asarray(w_attn, dtype=np.float32)
    b_attn = np.asarray(b_attn, dtype=np.float32)
    w_proj = np.asarray(w_proj, dtype=np.float32)
    assert not np.any(b_attn), "kernel assumes zero qkv bias"

    # q/k channel permutation: evens block then odds block, head-major
    qch = np.arange(H * HD).reshape(H, 32, 2)
    q_perm = np.concatenate([qch[:, :, 0].reshape(-1), qch[:, :, 1].reshape(-1)])
    kch = H * HD + np.arange(KV * HD).reshape(KV, 32, 2)
    k_perm = np.concatenate([kch[:, :, 0].reshape(-1), kch[:, :, 1].reshape(-1)])
    wq_f = np.ascontiguousarray(w_attn[q_perm].T)              # (1024, 1024)
    wk_f = np.ascontiguousarray(w_attn[k_perm].T)              # (1024, 256)
    wv_f = np.ascontiguousarray(w_attn[(H + KV) * HD:].T)      # (1024, 256)
    wp_h = np.ascontiguousarray(
        w_proj.T.reshape(8, 128, C).transpose(1, 0, 2)).astype(BF)

    if USE_FP8:
        def wpack(w, width):
            hi, lo = _f8_hilo(w.reshape(8, 128, width) * SW)
            # (k, p, 2, width) -> (p, k, 2, width)
            return np.ascontiguousarray(
                np.stack([hi, lo], axis=2).transpose(1, 0, 2, 3))
        wq_h = wpack(wq_f, 1024)
        wk_h = wpack(wk_f, 256)
        wv_h = wpack(wv_f, 256)
        tab_scale = DESCALE
    else:
        wq_h = np.ascontiguousarray(
            wq_f.reshape(8, 128, 1024).transpose(1, 0, 2)).astype(BF)
        wk_h = np.ascontiguousarray(
            wk_f.reshape(8, 128, 256).transpose(1, 0, 2)).astype(BF)
        wv_h = np.ascontiguousarray(
            wv_f.reshape(8, 128, 256).transpose(1, 0, 2)).astype(BF)
        tab_scale = 1.0

    cos4 = np.tile(freqs_cos.T, (4, 1)).astype(np.float32) * tab_scale  # (128, T)
    sin4 = np.tile(freqs_sin.T, (4, 1)).astype(np.float32) * tab_scale

    in_maps = []
    for core in range(8):
        b, h = divmod(core, 2)
        t0 = h * TL
        xs = np.zeros((TH, C), dtype=np.float32)
        lo_ = max(0, t0 - WIN)
        xs[TH - (t0 + TL - lo_):] = x[b, lo_:t0 + TL]
        if USE_FP8:
            xk = xs.T.reshape(8, 128, TH) * SX
            x_hi, x_lo = _f8_hilo(xk)
            xh = np.ascontiguousarray(np.stack([x_hi, x_lo], axis=2))
        else:
            xh = np.ascontiguousarray(xs.T.reshape(8, 128, TH)).astype(BF)
        vbv = np.zeros((1, 640), dtype=np.float32)
        if h == 0:
            vbv[0, :WIN] = NEG
        cpad = np.zeros((128, TH), dtype=np.float32)
        spad = np.zeros((128, TH), dtype=np.float32)
        cpad[:, TH - (t0 + TL - lo_):] = cos4[:, lo_:t0 + TL]
        spad[:, TH - (t0 + TL - lo_):] = sin4[:, lo_:t0 + TL]
        in_maps.append({
            "xf": xh,
            "wq": wq_h, "wk": wk_h, "wv": wv_h, "wp": wp_h,
            "cq": np.ascontiguousarray(
                np.stack([cos4[:, t0:t0 + TL],
                          sin4[:, t0:t0 + TL]], axis=1)).astype(BF),
            "ck": np.ascontiguousarray(
                np.stack([cpad, spad], axis=1)).astype(BF),
            "vb": vbv.astype(BF),
        })

    return in_maps


def kernel(**inputs):
    in_maps = prepare_in_maps(**inputs)
    nc = _get_program()
    res = run_bass_kernel_spmd(nc, in_maps, list(range(8)))
    return _gather(res, np.asarray(inputs["b_proj"], dtype=np.float32))


def _gather(res, b_proj):
    out = np.empty((B, T, C), dtype=np.float32)
    for core in range(8):
        b, h = divmod(core, 2)
        out[b, h * TL:(h + 1) * TL] = res.results[core]["out"]
    if np.any(b_proj):
        out += b_proj
    return out
